# revision 5
# baseline (speedup 1.0000x reference)
"""Trainium2 Bass kernel for nn_BasicMultiUpdateBlock (RAFT-style update block).

Sharding: pure data-parallel over batch — B=8 images, one per NeuronCore.
Each core runs a 3-phase line-buffered implicit-GEMM pipeline:
  Phase A: motion encoder (convc1/convc2/convf1/convf2/enc) -> motion[64,H,W] DRAM scratch
  Phase B: ConvGRU (convz/convr/convq, dil=2) -> h[128,H,W] (output)
  Phase C: FlowHead + mask head -> delta_flow[1,H,W], mask[144,H,W]

Feature maps live in SBUF as [C(partitions), row, W+pad(cols)]; convs are
per-output-row matmuls with channels on the contraction dim, taps realized by
column shifts into x-padded row buffers and row-indexed circular buffers.
Small-C convs pack 2-3 x-shifted copies of the input into partition blocks so
all kx taps of one ky fold into a single matmul.
"""

import numpy as np

H, W = 160, 320
WB = W + 4          # padded buffer row width (dil<=2 -> pad 2 per side)
COR, MID, MOT, HID = 36, 32, 64, 128
F32 = None          # filled lazily (mybir import inside functions)

_NC_CACHE = {}


# ---------------------------------------------------------------------------
# Weight packing layout (shared between host packing and device builder)
# ---------------------------------------------------------------------------
def _layout():
    """Returns (entries, total_cols). entries: name -> (col, K, M, base_part)."""
    entries = {}
    col = 0

    def add(name, K, M, base=0):
        nonlocal col
        entries[name] = (col, K, M, base)
        col += M

    add("c1", COR, MID)
    for ky in range(3):
        add(f"c2_{ky}", 96, MID)
    add("f1", 49, MID)
    for ky in range(3):
        add(f"f2_{ky}", 96, MID)
    for ky in range(3):
        add(f"encp_{ky}", 128, 63)
    for ky in range(3):
        add(f"encs_{ky}", 64, 63, base=64)
    for cv in ("z", "r", "q"):
        for ky in range(3):
            for kx in range(3):
                add(f"{cv}_n0_{ky}{kx}", 128, 128)
        for ky in range(3):
            add(f"{cv}_mp_{ky}", 128, 128)
        for ky in range(3):
            add(f"{cv}_ms_{ky}", 64, 128, base=64)
        for ky in range(3):
            for kx in range(3):
                add(f"{cv}_in_{ky}{kx}", 128, 128)
    for ky in range(3):
        for kx in range(3):
            add(f"fh1_{ky}{kx}", 128, 128)
    add("fh2", 128, 1)
    for ky in range(3):
        for kx in range(3):
            add(f"m1_{ky}{kx}", 128, 128)
    add("m2a", 128, 128)
    add("m2b", 128, 16)
    return entries, col


_BIAS_COLS = {
    "bc1": MID, "bc2": MID, "bf1": MID, "bf2": MID, "benc": 63,
    "bz": HID, "br": HID, "bq": HID, "bfh2": 1, "bm1": HID,
    "bm2a": 128, "bm2b": 16,
}
_BIAS_ORDER = list(_BIAS_COLS)


def _pack_weights(params):
    """Host-side: produce wpack [128, NW] and bpack [128, NB] float32."""
    p = {k: np.asarray(v, np.float32) for k, v in params.items()}
    entries, nw = _layout()
    wpack = np.zeros((128, nw), np.float32)

    def put(name, block):
        col, K, M, base = entries[name]
        assert block.shape == (K, M), (name, block.shape, (K, M))
        wpack[base:base + K, col:col + M] = block

    put("c1", p["convc1_w"][:, :, 0, 0].T)                       # [36,32]
    for ky in range(3):
        w = p["convc2_w"]                                        # [32,32,3,3]
        put(f"c2_{ky}", np.concatenate([w[:, :, ky, kx].T for kx in range(3)], 0))
    wf1 = p["convf1_w"]                                          # [32,1,7,7]
    put("f1", wf1[:, 0, :, :].reshape(MID, 49).T)                # rows ky*7+kx
    for ky in range(3):
        w = p["convf2_w"]
        put(f"f2_{ky}", np.concatenate([w[:, :, ky, kx].T for kx in range(3)], 0))
    we = p["enc_w"]                                              # [63,64,3,3]
    for ky in range(3):
        put(f"encp_{ky}", np.concatenate(
            [we[:, 0:32, ky, 0].T, we[:, 32:64, ky, 0].T,
             we[:, 0:32, ky, 1].T, we[:, 32:64, ky, 1].T], 0))
        put(f"encs_{ky}", np.concatenate(
            [we[:, 0:32, ky, 2].T, we[:, 32:64, ky, 2].T], 0))
    for cv, wname in (("z", "convz_w"), ("r", "convr_w"), ("q", "convq_w")):
        w = p[wname]                                             # [128,320,3,3]
        for ky in range(3):
            for kx in range(3):
                put(f"{cv}_n0_{ky}{kx}", w[:, 0:128, ky, kx].T)
                put(f"{cv}_in_{ky}{kx}", w[:, 192:320, ky, kx].T)
            put(f"{cv}_mp_{ky}", np.concatenate(
                [w[:, 128:192, ky, 0].T, w[:, 128:192, ky, 1].T], 0))
            put(f"{cv}_ms_{ky}", w[:, 128:192, ky, 2].T)
    w = p["fh1_w"]
    for ky in range(3):
        for kx in range(3):
            put(f"fh1_{ky}{kx}", w[:, :, ky, kx].T)
    put("fh2", p["fh2_w"][:, :, 0, 0].T)                         # [128,1]
    w = p["mask1_w"]
    for ky in range(3):
        for kx in range(3):
            put(f"m1_{ky}{kx}", w[:, :, ky, kx].T)
    wm2 = p["mask2_w"][:, :, 0, 0].T * 0.25                      # [128,144]
    put("m2a", wm2[:, 0:128])
    put("m2b", wm2[:, 128:144])

    bpack = np.zeros((128, len(_BIAS_ORDER)), np.float32)
    bvals = {
        "bc1": p["convc1_b"], "bc2": p["convc2_b"], "bf1": p["convf1_b"],
        "bf2": p["convf2_b"], "benc": p["enc_b"], "bz": p["convz_b"],
        "br": p["convr_b"], "bq": p["convq_b"], "bfh2": p["fh2_b"],
        "bm1": p["mask1_b"],
        "bm2a": p["mask2_b"][0:128] * 0.25, "bm2b": p["mask2_b"][128:144] * 0.25,
    }
    for i, name in enumerate(_BIAS_ORDER):
        v = bvals[name]
        bpack[0:len(v), i] = v
    return wpack, bpack


# ---------------------------------------------------------------------------
# Device program
# ---------------------------------------------------------------------------
def _build(Himg):
    import concourse.bacc as bacc
    import concourse.bass as bass
    import concourse.mybir as mybir
    import concourse.tile as tile

    F32 = mybir.dt.float32
    AF = mybir.ActivationFunctionType
    entries, NW = _layout()
    NB = len(_BIAS_ORDER)

    nc = bacc.Bacc(None, target_bir_lowering=False)
    net0p = nc.dram_tensor("net0p", [128, Himg, WB], F32, kind="ExternalInput")
    inpp = nc.dram_tensor("inpp", [128, Himg, WB], F32, kind="ExternalInput")
    corr = nc.dram_tensor("corr", [COR, Himg, W], F32, kind="ExternalInput")
    flowp = nc.dram_tensor("flowp", [1, Himg + 6, W + 6], F32, kind="ExternalInput")
    wdram = nc.dram_tensor("wpack", [128, NW], F32, kind="ExternalInput")
    bdram = nc.dram_tensor("bpack", [128, NB], F32, kind="ExternalInput")
    hout = nc.dram_tensor("h", [128, Himg, W], F32, kind="ExternalOutput")
    dfout = nc.dram_tensor("dflow", [1, Himg, W], F32, kind="ExternalOutput")
    mkout = nc.dram_tensor("mask", [144, Himg, W], F32, kind="ExternalOutput")
    motion = nc.dram_tensor("motion", [64, Himg, W], F32)

    with tile.TileContext(nc) as tc:
        with tc.tile_pool(name="wpool", bufs=1) as wpool:
            wsb = wpool.tile([128, NW], F32)
            nc.sync.dma_start(wsb[:], wdram[:])
            bsb = wpool.tile([128, NB], F32)
            nc.sync.dma_start(bsb[:], bdram[:])

            def W_(name):
                col, K, M, base = entries[name]
                return wsb[base:base + K, col:col + M]

            def B_(name):
                i = _BIAS_ORDER.index(name)
                return bsb[0:_BIAS_COLS[name], i:i + 1]

            # ------------------------- Phase A -------------------------
            with tc.tile_pool(name="pa", bufs=1) as pa, \
                 tc.tile_pool(name="pa2", bufs=3) as pa2, \
                 tc.tile_pool(name="psA", bufs=1, space="PSUM") as psA:
                cor1b = pa.tile([128, 3, WB], F32)   # 0-31@p2, 32-63@p1, 64-95@p0
                flo1b = pa.tile([128, 3, WB], F32)
                ecb = pa.tile([128, 3, WB], F32)     # c2@p2, f2@p2, c2@p1, f2@p1
                nc.vector.memset(cor1b[:], 0.0)
                nc.vector.memset(flo1b[:], 0.0)
                nc.vector.memset(ecb[:], 0.0)
                for t in range(Himg + 2):
                    if t < Himg:
                        s = t % 3
                        crow = pa2.tile([COR, W], F32, tag="crow")
                        nc.sync.dma_start(crow[:], corr[:, t, :])
                        f1c = pa2.tile([49, W], F32, tag="f1c")
                        src = bass.AP(flowp, offset=t * (W + 6),
                                      ap=[[W + 6, 7], [1, 7], [1, W]])
                        nc.sync.dma_start(f1c[:], src)
                        c1p = psA.tile([MID, W], F32, tag="c1p", bufs=1)
                        nc.tensor.matmul(c1p[:], W_("c1"), crow[:],
                                         start=True, stop=True)
                        nc.scalar.activation(cor1b[0:32, s, 2:2 + W], c1p[:],
                                             AF.Relu, bias=B_("bc1"))
                        nc.sync.dma_start(cor1b[32:64, s, 1:1 + W],
                                          cor1b[0:32, s, 2:2 + W])
                        nc.sync.dma_start(cor1b[64:96, s, 0:W],
                                          cor1b[0:32, s, 2:2 + W])
                        f1p = psA.tile([MID, W], F32, tag="f1p", bufs=1)
                        nc.tensor.matmul(f1p[:], W_("f1"), f1c[:],
                                         start=True, stop=True)
                        nc.scalar.activation(flo1b[0:32, s, 2:2 + W], f1p[:],
                                             AF.Relu, bias=B_("bf1"))
                        nc.sync.dma_start(flo1b[32:64, s, 1:1 + W],
                                          flo1b[0:32, s, 2:2 + W])
                        nc.sync.dma_start(flo1b[64:96, s, 0:W],
                                          flo1b[0:32, s, 2:2 + W])
                    if 1 <= t <= Himg:
                        j = t - 1
                        s = j % 3
                        taps = [ky for ky in range(3) if 0 <= j + ky - 1 < Himg]
                        c2p = psA.tile([MID, W], F32, tag="c2p", bufs=1)
                        for i, ky in enumerate(taps):
                            nc.tensor.matmul(
                                c2p[:], W_(f"c2_{ky}"),
                                cor1b[0:96, (j + ky - 1) % 3, 1:1 + W],
                                start=(i == 0), stop=(i == len(taps) - 1))
                        nc.scalar.activation(ecb[0:32, s, 2:2 + W], c2p[:],
                                             AF.Relu, bias=B_("bc2"))
                        nc.sync.dma_start(ecb[64:96, s, 1:1 + W],
                                          ecb[0:32, s, 2:2 + W])
                        f2p = psA.tile([MID, W], F32, tag="f2p", bufs=1)
                        for i, ky in enumerate(taps):
                            nc.tensor.matmul(
                                f2p[:], W_(f"f2_{ky}"),
                                flo1b[0:96, (j + ky - 1) % 3, 1:1 + W],
                                start=(i == 0), stop=(i == len(taps) - 1))
                        nc.scalar.activation(ecb[32:64, s, 2:2 + W], f2p[:],
                                             AF.Relu, bias=B_("bf2"))
                        nc.sync.dma_start(ecb[96:128, s, 1:1 + W],
                                          ecb[32:64, s, 2:2 + W])
                    if t >= 2:
                        k = t - 2
                        taps = [ky for ky in range(3) if 0 <= k + ky - 1 < Himg]
                        n = 2 * len(taps)
                        ep = psA.tile([63, W], F32, tag="ep", bufs=2)
                        i = 0
                        for ky in taps:
                            ss = (k + ky - 1) % 3
                            nc.tensor.matmul(ep[:], W_(f"encp_{ky}"),
                                             ecb[0:128, ss, 1:1 + W],
                                             start=(i == 0), stop=(i == n - 1))
                            i += 1
                            nc.tensor.matmul(ep[:], W_(f"encs_{ky}"),
                                             ecb[64:128, ss, 2:2 + W],
                                             start=False, stop=(i == n - 1))
                            i += 1
                        mrow = pa2.tile([64, W], F32, tag="mrow")
                        nc.scalar.activation(mrow[0:63, :], ep[:],
                                             AF.Relu, bias=B_("benc"))
                        nc.sync.dma_start(mrow[63:64, :],
                                          flowp[0:1, k + 3, 3:3 + W])
                        nc.sync.dma_start(motion[:, k, :], mrow[:])

            tc.strict_bb_all_engine_barrier()

            # ------------------------- Phase B -------------------------
            SB = 8
            with tc.tile_pool(name="pb", bufs=1) as pb, \
                 tc.tile_pool(name="pb2", bufs=3) as pb2, \
                 tc.tile_pool(name="psB", bufs=2, space="PSUM") as psB:
                n0b = pb.tile([128, SB, WB], F32)
                inb = pb.tile([128, SB, WB], F32)
                mb = pb.tile([128, SB, WB], F32)
                rnb = pb.tile([128, 5, WB], F32)
                zb = pb.tile([128, 3, W], F32)
                nc.vector.memset(mb[:], 0.0)
                nc.vector.memset(rnb[:], 0.0)

                def load_row(rr):
                    sl = rr % SB
                    nc.sync.dma_start(n0b[:, sl, :], net0p[:, rr, :])
                    nc.sync.dma_start(inb[:, sl, :], inpp[:, rr, :])
                    nc.sync.dma_start(mb[0:64, sl, 4:4 + W], motion[:, rr, :])
                    nc.sync.dma_start(mb[64:128, sl, 2:2 + W], motion[:, rr, :])

                def gru_mms(ps, cv, y):
                    # first contraction chunk: net0 for z/r, rn (=r*net0) for q
                    taps = [ky for ky in range(3) if 0 <= y + 2 * (ky - 1) < Himg]
                    n = 8 * len(taps)
                    i = 0
                    for ky in taps:
                        r0 = y + 2 * (ky - 1)
                        sl = r0 % SB
                        for kx in range(3):
                            if cv == "q":
                                fb, ssl = rnb, r0 % 5
                            else:
                                fb, ssl = n0b, sl
                            nc.tensor.matmul(
                                ps[:], W_(f"{cv}_n0_{ky}{kx}"),
                                fb[0:128, ssl, 2 * kx:2 * kx + W],
                                start=(i == 0), stop=(i == n - 1))
                            i += 1
                        nc.tensor.matmul(ps[:], W_(f"{cv}_mp_{ky}"),
                                         mb[0:128, sl, 2:2 + W],
                                         start=False, stop=(i == n - 1))
                        i += 1
                        nc.tensor.matmul(ps[:], W_(f"{cv}_ms_{ky}"),
                                         mb[64:128, sl, 4:4 + W],
                                         start=False, stop=(i == n - 1))
                        i += 1
                        for kx in range(3):
                            nc.tensor.matmul(ps[:], W_(f"{cv}_in_{ky}{kx}"),
                                             inb[0:128, sl, 2 * kx:2 * kx + W],
                                             start=False, stop=(i == n - 1))
                            i += 1

                load_row(0)
                if Himg > 1:
                    load_row(1)
                for t in range(Himg + 2):
                    if t + 2 < Himg:
                        load_row(t + 2)
                    if t < Himg:
                        zp = psB.tile([HID, W], F32, tag="zp")
                        gru_mms(zp, "z", t)
                        rp = psB.tile([HID, W], F32, tag="rp")
                        gru_mms(rp, "r", t)
                        nc.scalar.activation(zb[:, t % 3, :], zp[:],
                                             AF.Sigmoid, bias=B_("bz"))
                        rrow = pb2.tile([HID, W], F32, tag="rrow")
                        nc.scalar.activation(rrow[:], rp[:],
                                             AF.Sigmoid, bias=B_("br"))
                        nc.vector.tensor_mul(rnb[0:128, t % 5, 2:2 + W],
                                             rrow[:],
                                             n0b[0:128, t % SB, 2:2 + W])
                    if t >= 2:
                        y = t - 2
                        qp = psB.tile([HID, W], F32, tag="qp")
                        gru_mms(qp, "q", y)
                        qrow = pb2.tile([HID, W], F32, tag="qrow")
                        nc.scalar.activation(qrow[:], qp[:],
                                             AF.Tanh, bias=B_("bq"))
                        n0s = n0b[0:128, y % SB, 2:2 + W]
                        hrow = pb2.tile([HID, W], F32, tag="hrow")
                        nc.vector.tensor_sub(hrow[:], qrow[:], n0s)
                        nc.vector.tensor_mul(hrow[:], hrow[:], zb[:, y % 3, :])
                        nc.vector.tensor_add(hrow[:], hrow[:], n0s)
                        nc.sync.dma_start(hout[:, y, :], hrow[:])

            tc.strict_bb_all_engine_barrier()

            # ------------------------- Phase C -------------------------
            with tc.tile_pool(name="pc", bufs=1) as pc, \
                 tc.tile_pool(name="pc2", bufs=3) as pc2, \
                 tc.tile_pool(name="psC", bufs=1, space="PSUM") as psC:
                hb = pc.tile([128, SB, WB], F32)
                nc.vector.memset(hb[:], 0.0)

                def load_h(rr):
                    nc.sync.dma_start(hb[:, rr % SB, 2:2 + W], hout[:, rr, :])

                load_h(0)
                if Himg > 1:
                    load_h(1)
                for t in range(Himg + 2):
                    if t + 2 < Himg:
                        load_h(t + 2)
                    if t < 2:
                        continue
                    y = t - 2
                    # FlowHead: fh1 (dil2) + fh2 (1x1)
                    taps2 = [ky for ky in range(3) if 0 <= y + 2 * (ky - 1) < Himg]
                    dp = psC.tile([HID, W], F32, tag="dp", bufs=2)
                    n = 3 * len(taps2)
                    i = 0
                    for ky in taps2:
                        sl = (y + 2 * (ky - 1)) % SB
                        for kx in range(3):
                            nc.tensor.matmul(dp[:], W_(f"fh1_{ky}{kx}"),
                                             hb[0:128, sl, 2 * kx:2 * kx + W],
                                             start=(i == 0), stop=(i == n - 1))
                            i += 1
                    drow = pc2.tile([HID, W], F32, tag="drow")
                    nc.scalar.activation(drow[:], dp[:], AF.Relu)
                    dfp = psC.tile([1, W], F32, tag="dfp", bufs=1)
                    nc.tensor.matmul(dfp[:], W_("fh2"), drow[:],
                                     start=True, stop=True)
                    dfrow = pc2.tile([1, W], F32, tag="dfrow")
                    nc.scalar.activation(dfrow[:], dfp[:], AF.Identity,
                                         bias=B_("bfh2"))
                    nc.sync.dma_start(dfout[0:1, y, :], dfrow[:])
                    # Mask head: mask1 (dil1) + mask2 (1x1, pre-scaled 0.25)
                    taps1 = [ky for ky in range(3) if 0 <= y + ky - 1 < Himg]
                    mp = psC.tile([HID, W], F32, tag="mp", bufs=2)
                    n = 3 * len(taps1)
                    i = 0
                    for ky in taps1:
                        sl = (y + ky - 1) % SB
                        for kx in range(3):
                            nc.tensor.matmul(mp[:], W_(f"m1_{ky}{kx}"),
                                             hb[0:128, sl, 1 + kx:1 + kx + W],
                                             start=(i == 0), stop=(i == n - 1))
                            i += 1
                    m1row = pc2.tile([HID, W], F32, tag="m1row")
                    nc.scalar.activation(m1row[:], mp[:], AF.Relu, bias=B_("bm1"))
                    map_ = psC.tile([128, W], F32, tag="map", bufs=1)
                    nc.tensor.matmul(map_[:], W_("m2a"), m1row[:],
                                     start=True, stop=True)
                    mka = pc2.tile([128, W], F32, tag="mka")
                    nc.scalar.activation(mka[:], map_[:], AF.Identity,
                                         bias=B_("bm2a"))
                    nc.sync.dma_start(mkout[0:128, y, :], mka[:])
                    mbp = psC.tile([16, W], F32, tag="mbp", bufs=1)
                    nc.tensor.matmul(mbp[:], W_("m2b"), m1row[:],
                                     start=True, stop=True)
                    mkb = pc2.tile([16, W], F32, tag="mkb")
                    nc.scalar.activation(mkb[:], mbp[:], AF.Identity,
                                         bias=B_("bm2b"))
                    nc.sync.dma_start(mkout[128:144, y, :], mkb[:])

    nc.finalize()
    return nc


def _get_nc(Himg):
    if Himg not in _NC_CACHE:
        _NC_CACHE[Himg] = _build(Himg)
    return _NC_CACHE[Himg]


# ---------------------------------------------------------------------------
# Host entry point
# ---------------------------------------------------------------------------
def _make_in_maps(net0, inp, corr, flow, params, Himg):
    wpack, bpack = _pack_weights(params)
    in_maps = []
    for b in range(net0.shape[0]):
        in_maps.append({
            "net0p": np.ascontiguousarray(
                np.pad(np.asarray(net0[b], np.float32), ((0, 0), (0, 0), (2, 2)))),
            "inpp": np.ascontiguousarray(
                np.pad(np.asarray(inp[b], np.float32), ((0, 0), (0, 0), (2, 2)))),
            "corr": np.ascontiguousarray(np.asarray(corr[b], np.float32)),
            "flowp": np.ascontiguousarray(
                np.pad(np.asarray(flow[b], np.float32), ((0, 0), (3, 3), (3, 3)))),
            "wpack": wpack,
            "bpack": bpack,
        })
    return in_maps


def run(net0, inp, corr, flow, params, trace=False, tmpdir=None):
    from concourse.bass_utils import run_bass_kernel_spmd
    Himg = net0.shape[2]
    nc = _get_nc(Himg)
    B = net0.shape[0]
    in_maps = _make_in_maps(net0, inp, corr, flow, params, Himg)
    res = run_bass_kernel_spmd(nc, in_maps, core_ids=list(range(B)),
                               trace=trace, tmpdir=tmpdir)
    h = np.stack([res.results[b]["h"] for b in range(B)])
    df = np.stack([res.results[b]["dflow"] for b in range(B)])
    mask = np.stack([res.results[b]["mask"] for b in range(B)])
    return (h, df, mask), res


def kernel(net0, inp, corr, flow, params):
    (h, df, mask), _ = run(net0, inp, corr, flow, params)
    return h, df, mask
